# revision 1
# baseline (speedup 1.0000x reference)
"""KV-cached multi-head attention on 8 Trainium2 NeuronCores.

Sharding: 4-way batch (data parallel) x 2-way heads (tensor parallel).
Core c handles batch b = c//2 and head-half h2 = c%2 (8 of 16 heads).
Each core: Q/K/V projections (column-sharded), 8-head causal attention
against the concatenated KV cache, and a row-sharded out-projection
partial. The two partials per batch are summed on the host (+ bo).

Device kernel layout choices:
  - Projections computed in f32r (tf32-class, 1 cyc/row on PE).
  - Q^T/K^T produced head-major [head_dim, tokens]; scores computed
    TRANSPOSED (S^T = K^T.T @ Q^T per 128-key tile) so no P transpose
    is ever needed; exp on ACT (no max subtraction: |scores| <= ~8 for
    this distribution, fp32 exp is safe); softmax denominator via a
    ones-column matmul on PE; PV accumulates O^T = V.T @ P^T directly.
  - P / V / Q^T / K^T in bf16 (PE 1 cyc/row, fp32 PSUM accumulation).
  - Causal mask handled structurally: per 512-query chunk only the
    needed key tiles are computed; the 4 diagonal key tiles use a
    reduced query range plus one 128x128 triangular bf16 multiply.
"""

import sys

sys.path.insert(0, "/opt/trn_rl_repo")

import numpy as np
import ml_dtypes

import concourse.bass as bass  # noqa: F401  (registers AP types)
import concourse.mybir as mybir
import concourse.tile as tile
from concourse import bacc
from concourse.bass_utils import run_bass_kernel_spmd

F32 = mybir.dt.float32
F32R = mybir.dt.float32r
BF16 = mybir.dt.bfloat16
BF = ml_dtypes.bfloat16

D = 2048          # model dim
SQ = 1024         # new tokens per batch
SC = 1024         # cached tokens
SKV = SC + SQ     # total keys
HD = 128          # head dim
HLOC = 8          # heads per core
DH = HLOC * HD    # per-core projected dim (1024)
KC = 17           # contraction chunks (2048 + bias row, padded to 17*128)
KAUG = KC * 128   # 2176
NCORES = 8

EXP = mybir.ActivationFunctionType.Exp


def _emit(tc, nc, prm):
    P = 128

    xq_r = prm["xq"].rearrange("(t p) n -> p t n", p=P)
    xk_r = prm["xk"].rearrange("(t p) n -> p t n", p=P)

    with tc.tile_pool(name="res", bufs=1) as res:
        qt = [res.tile([P, SQ], BF16, name=f"qt{h}", tag=f"qt{h}") for h in range(HLOC)]
        kt = [res.tile([P, SKV], BF16, name=f"kt{h}", tag=f"kt{h}") for h in range(HLOC)]
        vv = [res.tile([P, DH], BF16, name=f"vv{t}", tag=f"vv{t}") for t in range(16)]
        tri = res.tile([P, P], BF16, name="tri", tag="tri")
        ones = res.tile([P, 1], BF16, name="ones", tag="ones")

        nc.sync.dma_start(tri[:], prm["tri"][:])
        nc.vector.memset(ones[:], 1.0)
        # KV cache loads (already bf16, pre-transposed/sliced on host)
        for h in range(HLOC):
            nc.sync.dma_start(kt[h][:, 0:SC], prm["ckt"][P * h : P * (h + 1), :])
        for t in range(8):
            nc.sync.dma_start(vv[t][:], prm["cv"][P * t : P * (t + 1), :])

        # ---------------- projections ----------------
        with (
            tc.tile_pool(name="pjx", bufs=2) as pjx,
            tc.tile_pool(name="pjw", bufs=4) as pjw,
            tc.tile_pool(name="pjps", bufs=1, space="PSUM") as pjps,
        ):
            # V: out[tok_tile, dout] = xv.T @ wv   (natural [tok, dh] layout)
            for cd in range(2):
                ps_t = [
                    pjps.tile([P, 512], F32, name=f"vps{cd}_{m}", tag=f"ps{m}")
                    for m in range(8)
                ]
                for k in range(KC):
                    xv_t = pjw.tile([P, SQ], F32R, name=f"xv{cd}_{k}", tag="xvk")
                    nc.sync.dma_start(xv_t[:], prm["xv"][P * k : P * (k + 1), :])
                    wv_t = pjw.tile([P, 512], F32R, name=f"wv{cd}_{k}", tag="wvk")
                    nc.sync.dma_start(
                        wv_t[:], prm["wv"][P * k : P * (k + 1), 512 * cd : 512 * (cd + 1)]
                    )
                    for m in range(8):
                        nc.tensor.matmul(
                            ps_t[m][:],
                            xv_t[:, P * m : P * (m + 1)],
                            wv_t[:],
                            start=(k == 0),
                            stop=(k == KC - 1),
                        )
                for m in range(8):
                    nc.scalar.copy(vv[8 + m][:, 512 * cd : 512 * (cd + 1)], ps_t[m][:])

            # K then Q: out[dout_tile, tok] = w.T @ x  (transposed layout)
            for name_x, xr, name_w, dest, col0 in (
                ("xk", xk_r, "wk", kt, SC),
                ("xq", xq_r, "wq", qt, 0),
            ):
                for c in range(2):
                    xc = pjx.tile([P, KC, 512], F32R, name=f"{name_x}c{c}", tag="pjx")
                    nc.sync.dma_start(xc[:], xr[:, :, 512 * c : 512 * (c + 1)])
                    ps_m = [
                        pjps.tile([P, 512], F32, name=f"{name_w}ps{c}_{m}", tag=f"ps{m}")
                        for m in range(8)
                    ]
                    for k in range(KC):
                        w_t = pjw.tile([P, DH], F32R, name=f"{name_w}{c}_{k}", tag="wk")
                        nc.sync.dma_start(w_t[:], prm[name_w][P * k : P * (k + 1), :])
                        for m in range(8):
                            nc.tensor.matmul(
                                ps_m[m][:],
                                w_t[:, P * m : P * (m + 1)],
                                xc[:, k, :],
                                start=(k == 0),
                                stop=(k == KC - 1),
                            )
                    for m in range(8):
                        nc.scalar.copy(
                            dest[m][:, col0 + 512 * c : col0 + 512 * c + 512], ps_m[m][:]
                        )

        # ---------------- attention ----------------
        with tc.tile_pool(name="at_p", bufs=1) as at_p:
          at = [
              at_p.tile([P, SQ], F32R, name=f"at{t}", tag=f"at{t}") for t in range(HLOC)
          ]
          with (
            tc.tile_pool(name="stps", bufs=4, space="PSUM") as stps,
            tc.tile_pool(name="ops", bufs=2, space="PSUM") as ops,
            tc.tile_pool(name="dps", bufs=2, space="PSUM") as dps,
            tc.tile_pool(name="ptp", bufs=8) as ptp,
            tc.tile_pool(name="bcp", bufs=3) as bcp,
          ):
            for h in range(HLOC):
                for c in range(2):
                    n_full = 8 + 4 * c
                    n_kv = n_full + 4
                    q_sl = slice(512 * c, 512 * (c + 1))
                    o_ps = ops.tile([P, 512], F32, name=f"o{h}_{c}", tag="o")
                    d_ps = dps.tile([1, 512], F32, name=f"d{h}_{c}", tag="d")
                    for g in range(n_kv):
                        j = g - n_full  # >= 0 on diagonal tiles
                        st = stps.tile([P, 512], F32, name=f"st{h}_{c}_{g}", tag="st")
                        pt = ptp.tile([P, 512], BF16, name=f"pt{h}_{c}_{g}", tag="pt")
                        if j < 0:
                            nc.tensor.matmul(
                                st[:], kt[h][:, P * g : P * (g + 1)], qt[h][:, q_sl],
                                start=True, stop=True,
                            )
                            nc.scalar.activation(pt[:], st[:], EXP)
                        else:
                            o0 = 128 * j
                            nc.tensor.matmul(
                                st[:, o0:512],
                                kt[h][:, P * g : P * (g + 1)],
                                qt[h][:, 512 * c + o0 : 512 * (c + 1)],
                                start=True, stop=True,
                            )
                            if o0:
                                nc.vector.memset(pt[:, 0:o0], 0.0)
                            nc.scalar.activation(pt[:, o0:512], st[:, o0:512], EXP)
                            nc.vector.tensor_mul(
                                pt[:, o0 : o0 + P], pt[:, o0 : o0 + P], tri[:]
                            )
                        nc.tensor.matmul(
                            o_ps[:], vv[g][:, P * h : P * (h + 1)], pt[:],
                            start=(g == 0), stop=(g == n_kv - 1),
                        )
                        nc.tensor.matmul(
                            d_ps[:], ones[:], pt[:],
                            start=(g == 0), stop=(g == n_kv - 1),
                        )
                    rec = bcp.tile([1, 512], F32, name=f"rec{h}_{c}", tag="rec")
                    nc.vector.reciprocal(rec[:], d_ps[:])
                    bc = bcp.tile([P, 512], F32, name=f"bc{h}_{c}", tag="bc")
                    nc.gpsimd.partition_broadcast(bc[:], rec[:])
                    nc.vector.tensor_mul(at[h][:, q_sl], o_ps[:], bc[:])

          # ---------------- out-projection ----------------
          with (
              tc.tile_pool(name="wop", bufs=3) as wop,
              tc.tile_pool(name="out_ps", bufs=4, space="PSUM") as out_ps,
              tc.tile_pool(name="outs", bufs=4) as outs,
          ):
              for m in range(16):
                  wo_t = wop.tile([P, 8, P], F32R, name=f"wo{m}", tag="wo")
                  nc.sync.dma_start(
                      wo_t[:], prm["wo"][m].rearrange("(t p) n -> p t n", p=P)
                  )
                  for c in range(2):
                      op = out_ps.tile([P, 512], F32, name=f"op{m}_{c}", tag="op")
                      for t in range(HLOC):
                          nc.tensor.matmul(
                              op[:], wo_t[:, t, :], at[t][:, 512 * c : 512 * (c + 1)],
                              start=(t == 0), stop=(t == HLOC - 1),
                          )
                      ob = outs.tile([P, 512], F32, name=f"ob{m}_{c}", tag="ob")
                      nc.scalar.copy(ob[:], op[:])
                      nc.sync.dma_start(
                          prm["outT"][P * m : P * (m + 1), 512 * c : 512 * (c + 1)],
                          ob[:],
                      )


def build():
    nc = bacc.Bacc(None, target_bir_lowering=False)
    prm = {}
    for n, shape, dt in (
        ("xq", [KAUG, SQ], F32R),
        ("xk", [KAUG, SQ], F32R),
        ("xv", [KAUG, SQ], F32R),
        ("wq", [KAUG, DH], F32R),
        ("wk", [KAUG, DH], F32R),
        ("wv", [KAUG, DH], F32R),
        ("wo", [16, DH, 128], F32R),
        ("ckt", [DH, SC], BF16),
        ("cv", [SC, DH], BF16),
        ("tri", [128, 128], BF16),
    ):
        prm[n] = nc.declare_dram_parameter(n, shape, dt, isOutput=False)
    prm["outT"] = nc.declare_dram_parameter("outT", [D, SQ], F32, isOutput=True)
    with tile.TileContext(nc) as tc:
        _emit(tc, nc, prm)
    nc.compile()
    return nc


def make_in_maps(query, key, value, cached_k, cached_v, Wq, bq, Wk, bk, Wv, bv, Wo, bo):
    """Per-core host prep: slice + transpose + bias-augment + casts."""
    s = float(np.sqrt(HD))
    tri = np.triu(np.ones((128, 128), dtype=np.float32)).astype(BF)

    def aug_x(x):  # [SQ, D] -> [KAUG, SQ] with ones row at 2048
        a = np.zeros((KAUG, SQ), dtype=np.float32)
        a[:D] = np.ascontiguousarray(x.T)
        a[D] = 1.0
        return a

    def aug_w(w, b):  # w [DH, D] (rows = out features), b [DH] -> [KAUG, DH]
        a = np.zeros((KAUG, DH), dtype=np.float32)
        a[:D] = np.ascontiguousarray(w.T)
        a[D] = b
        return a

    in_maps = []
    for c in range(NCORES):
        b, h2 = c // 2, c % 2
        hs = slice(DH * h2, DH * (h2 + 1))
        wo_s = np.ascontiguousarray(Wo[:, hs].T)  # [DH, D]
        in_maps.append(
            {
                "xq": aug_x(query[b]),
                "xk": aug_x(key[b]),
                "xv": aug_x(value[b]),
                "wq": aug_w(Wq[hs] / s, bq[hs] / s),
                "wk": aug_w(Wk[hs], bk[hs]),
                "wv": aug_w(Wv[hs], bv[hs]),
                "wo": np.ascontiguousarray(
                    wo_s.reshape(DH, 16, 128).transpose(1, 0, 2)
                ),
                "ckt": np.ascontiguousarray(cached_k[b][:, hs].T).astype(BF),
                "cv": np.ascontiguousarray(cached_v[b][:, hs]).astype(BF),
                "tri": tri,
            }
        )
    return in_maps


_NC_CACHE = []


def get_nc():
    if not _NC_CACHE:
        _NC_CACHE.append(build())
    return _NC_CACHE[0]


def assemble(results, bo):
    out = np.empty((4, SQ, D), dtype=np.float32)
    for b in range(4):
        acc = results[2 * b]["outT"] + results[2 * b + 1]["outT"]  # [D, SQ]
        out[b] = acc.T + bo[None, :]
    return out


def kernel(query, key, value, cached_k, cached_v, Wq, bq, Wk, bk, Wv, bv, Wo, bo):
    query = np.asarray(query, dtype=np.float32)
    key = np.asarray(key, dtype=np.float32)
    value = np.asarray(value, dtype=np.float32)
    cached_k = np.asarray(cached_k, dtype=np.float32)
    cached_v = np.asarray(cached_v, dtype=np.float32)
    Wq, bq = np.asarray(Wq, np.float32), np.asarray(bq, np.float32)
    Wk, bk = np.asarray(Wk, np.float32), np.asarray(bk, np.float32)
    Wv, bv = np.asarray(Wv, np.float32), np.asarray(bv, np.float32)
    Wo, bo = np.asarray(Wo, np.float32), np.asarray(bo, np.float32)

    nc = get_nc()
    in_maps = make_in_maps(
        query, key, value, cached_k, cached_v, Wq, bq, Wk, bk, Wv, bv, Wo, bo
    )
    res = run_bass_kernel_spmd(nc, in_maps, list(range(NCORES)))
    return assemble(res.results, bo)



# revision 5
# speedup vs baseline: 1.4516x; 1.4516x over previous
"""KV-cached multi-head attention on 8 Trainium2 NeuronCores.

Sharding: 4-way batch (data parallel) x 2-way heads (tensor parallel).
Core c handles batch b = c//2 and head-half h2 = c%2 (8 of 16 heads).
Each core: Q/K/V projections (column-sharded), 8-head causal attention
against the concatenated KV cache, and a row-sharded out-projection
partial. The two partials per batch are summed on the host (+ bo).

v2 kernel layout:
  - Q/K/V projections and the out-projection run as fp8e4m3 DoubleRow
    matmuls (256-wide contraction per instruction, 0.5 cyc/row) with a
    3-term error split: x = xh + xl (both fp8), W = wh + wl;
    acc = (xh+xl)@wh + xh@wl. Host pre-scales W by a power of two so
    fp8 quantization stays in the normal range; the PSUM->SBUF copy
    (DVE tensor_scalar) undoes the scale and adds the bias.
  - V projection carries its bias as an fp8 ones-row contraction pair
    built from tiny on-device tiles.
  - Attention (scores, exp, PV, softmax denominator) stays in bf16:
    S^T = K^T.T @ Q^T per 128-key tile (no transposes needed), exp on
    ACT over 2-tile pairs, O^T = V.T @ P^T, denominator via ones-column
    matmul. Causality handled structurally per 512-query chunk with
    triangular-mask multiplies on the 4 diagonal tiles; diagonal
    scores/PV/ones matmuls stream only the live query range (full-width
    pairs are ordered first/last so PSUM start/stop cover all columns).
  - A = O/d is split to fp8 hi+lo on DVE for the fp8 out-projection;
    out-projection (c=0) is interleaved into attention (c=1) to keep
    the PE fed while ACT works on exps.
"""

import sys

sys.path.insert(0, "/opt/trn_rl_repo")

import numpy as np
import ml_dtypes

import concourse.bass as bass  # noqa: F401  (registers AP types)
import concourse.mybir as mybir
import concourse.tile as tile
from concourse import bacc
from concourse.bass_utils import run_bass_kernel_spmd

F32 = mybir.dt.float32
F16 = mybir.dt.float16
BF16 = mybir.dt.bfloat16
F8 = mybir.dt.float8e4
BF = ml_dtypes.bfloat16
E4 = ml_dtypes.float8_e4m3

D = 2048          # model dim
SQ = 1024         # new tokens per batch
SC = 1024         # cached tokens
SKV = SC + SQ     # total keys
HD = 128          # head dim
HLOC = 8          # heads per core
DH = HLOC * HD    # per-core projected dim (1024)
KP = 8            # fp8 DoubleRow contraction pairs (8 * 256 = 2048)
NCORES = 8
P = 128

SIG_Q = 512.0     # host pre-scale on Wq/sqrt(hd)
SIG_K = 64.0
SIG_V = 64.0
SIG_O = 64.0

EXP = mybir.ActivationFunctionType.Exp
DR = mybir.MatmulPerfMode.DoubleRow
MULT = mybir.AluOpType.mult
ADD = mybir.AluOpType.add

TERMS = ((0, 0), (1, 0), (0, 1))  # (x hi/lo, w hi/lo) split terms


def _pair_seq(c):
    """Pair schedule for query chunk c: [(g_even, g_odd, o0_even, o0_odd)].
    Full-width pairs first and last so o_ps/d_ps start/stop cover all
    columns; diagonal pairs (reduced query range) in the middle."""
    n_full = 8 + 4 * c            # full 128-key tiles
    full = [(2 * i, 2 * i + 1, 0, 0) for i in range(n_full // 2)]
    diag = [
        (n_full, n_full + 1, 0, 128),
        (n_full + 2, n_full + 3, 256, 384),
    ]
    return full[:2] + diag + full[2:]


def _emit(tc, nc, prm):
    with tc.tile_pool(name="res", bufs=1) as res:
        qt = [res.tile([P, SQ], BF16, name=f"qt{h}", tag=f"qt{h}") for h in range(HLOC)]
        kt = [res.tile([P, SKV], BF16, name=f"kt{h}", tag=f"kt{h}") for h in range(HLOC)]
        vv = [res.tile([P, DH], BF16, name=f"vv{t}", tag=f"vv{t}") for t in range(16)]
        a2h = [res.tile([P, 2, SQ], F8, name=f"a2h{p}", tag=f"a2h{p}") for p in range(4)]
        a2l = [res.tile([P, 2, SQ], F8, name=f"a2l{p}", tag=f"a2l{p}") for p in range(4)]
        tri = res.tile([P, P], BF16, name="tri", tag="tri")
        ones = res.tile([P, 1], BF16, name="ones", tag="ones")
        bqv = res.tile([P, HLOC], F32, name="bqv", tag="bqv")
        bkv = res.tile([P, HLOC], F32, name="bkv", tag="bkv")
        xb = res.tile([P, 2, P], F8, name="xb", tag="xb")
        wvb = res.tile([P, 2, DH], F8, name="wvb", tag="wvb")

        nc.sync.dma_start(tri[:], prm["tri"][:])
        nc.sync.dma_start(bqv[:], prm["bq"][:])
        nc.sync.dma_start(bkv[:], prm["bk"][:])
        nc.vector.memset(ones[:], 1.0)
        nc.vector.memset(xb[:], 0.0)
        nc.vector.memset(xb[0:1, 0, :], 1.0)
        nc.vector.memset(wvb[:], 0.0)
        nc.sync.dma_start(wvb[0:1, 0, :], prm["bv8"][:])

        # ---------------- V projection ----------------
        with (
            tc.tile_pool(name="pjv", bufs=1) as pjv,
            tc.tile_pool(name="pjps", bufs=1, space="PSUM") as pjps,
        ):
            xv_t, wv_t = [], []
            for kp in range(KP):
                xv = pjv.tile([P, 2, 2, SQ], F8, name=f"xv{kp}", tag=f"xv{kp}")
                nc.sync.dma_start(xv[:], prm["xv"][kp])
                wv = pjv.tile([P, 2, 2, DH], F8, name=f"wv{kp}", tag=f"wv{kp}")
                nc.sync.dma_start(wv[:], prm["wv"][kp])
                xv_t.append(xv)
                wv_t.append(wv)
            for cd in range(2):
                ps_m = [
                    pjps.tile([P, 512], F32, name=f"vps{cd}_{m}", tag=f"ps{m}")
                    for m in range(8)
                ]
                csl = slice(512 * cd, 512 * (cd + 1))
                for kp in range(KP):
                    for m in range(8):
                        msl = slice(P * m, P * (m + 1))
                        for xs, ws in TERMS:
                            nc.tensor.matmul(
                                ps_m[m][:],
                                xv_t[kp][:, xs, :, msl],
                                wv_t[kp][:, ws, :, csl],
                                start=(kp == 0 and xs == 0 and ws == 0),
                                stop=False,
                                perf_mode=DR,
                            )
                for m in range(8):
                    nc.tensor.matmul(
                        ps_m[m][:], xb[:], wvb[:, :, csl],
                        start=False, stop=True, perf_mode=DR,
                    )
                for m in range(8):
                    nc.vector.tensor_scalar_mul(
                        vv[8 + m][:, csl], ps_m[m][:], 1.0 / SIG_V
                    )

        # cached KV (needed from attention onward)
        for h in range(HLOC):
            nc.sync.dma_start(kt[h][:, 0:SC], prm["ckt"][P * h : P * (h + 1), :])
        for t in range(8):
            nc.sync.dma_start(vv[t][:], prm["cv"][P * t : P * (t + 1), :])

        # ---------------- K projection ----------------
        def kq_proj(x_t, w_t, c, ps_pool, dest, sig, bias_t, nm):
            ps_m = [
                ps_pool.tile([P, 512], F32, name=f"p{nm}{c}_{m}", tag=f"ps{m}")
                for m in range(8)
            ]
            csl = slice(512 * c, 512 * (c + 1))
            for kp in range(KP):
                for m in range(8):
                    msl = slice(P * m, P * (m + 1))
                    for xs, ws in TERMS:
                        nc.tensor.matmul(
                            ps_m[m][:],
                            w_t[kp][:, ws, :, msl],
                            x_t[kp][:, xs, :, csl],
                            start=(kp == 0 and xs == 0 and ws == 0),
                            stop=(kp == KP - 1 and xs == 0 and ws == 1),
                            perf_mode=DR,
                        )
            for m in range(8):
                nc.vector.tensor_scalar(
                    dest[m], ps_m[m][:], 1.0 / sig, bias_t[:, m : m + 1],
                    op0=MULT, op1=ADD,
                )

        with (
            tc.tile_pool(name="pjk", bufs=1) as pjk,
            tc.tile_pool(name="pjps2", bufs=1, space="PSUM") as pjps2,
        ):
            xk_t, wk_t = [], []
            for kp in range(KP):
                xk = pjk.tile([P, 2, 2, SQ], F8, name=f"xk{kp}", tag=f"xk{kp}")
                nc.sync.dma_start(xk[:], prm["xk"][kp])
                wk = pjk.tile([P, 2, 2, DH], F8, name=f"wk{kp}", tag=f"wk{kp}")
                nc.sync.dma_start(wk[:], prm["wk"][kp])
                xk_t.append(xk)
                wk_t.append(wk)
            for c in range(2):
                kq_proj(
                    xk_t, wk_t, c, pjps2,
                    [kt[m][:, SC + 512 * c : SC + 512 * c + 512] for m in range(8)],
                    SIG_K, bkv, "k",
                )

        # ---------------- Q projection (both halves) ----------------
        with (
            tc.tile_pool(name="pjq", bufs=1) as pjq,
            tc.tile_pool(name="pjps3", bufs=1, space="PSUM") as pjps3,
        ):
            xq_t, wq_t = [], []
            for kp in range(KP):
                xq = pjq.tile([P, 2, 2, SQ], F8, name=f"xq{kp}", tag=f"xq{kp}")
                nc.sync.dma_start(xq[:], prm["xq"][kp])
                wq = pjq.tile([P, 2, 2, DH], F8, name=f"wq{kp}", tag=f"wq{kp}")
                nc.sync.dma_start(wq[:], prm["wq"][kp])
                xq_t.append(xq)
                wq_t.append(wq)
            for c in range(2):
                kq_proj(
                    xq_t, wq_t, c, pjps3,
                    [qt[m][:, 512 * c : 512 * (c + 1)] for m in range(8)],
                    SIG_Q, bqv, "q",
                )

        # out-projection weights (resident)
        wo_t = []
        for m in range(16):
            wo = res.tile([P, 2, 4, 2, P], F8, name=f"wo{m}", tag=f"wo{m}")
            nc.sync.dma_start(wo[:], prm["wo"][m])
            wo_t.append(wo)

        # ---------------- attention + out-projection ----------------
        with (
            tc.tile_pool(name="stps", bufs=2, space="PSUM") as stps,
            tc.tile_pool(name="ops", bufs=1, space="PSUM") as ops,
            tc.tile_pool(name="dps", bufs=1, space="PSUM") as dps,
            tc.tile_pool(name="aux", bufs=1, space="PSUM") as aux,
            tc.tile_pool(name="ptp", bufs=3) as ptp,
            tc.tile_pool(name="bcp", bufs=1) as bcp,
            tc.tile_pool(name="outs", bufs=4) as outs,
        ):
            def att(h, c):
                seq = _pair_seq(c)
                npair = len(seq)
                hsl = slice(P * h, P * (h + 1))
                q_hi = 512 * (c + 1)
                o_ps = ops.tile([P, 512], F32, name=f"o{h}_{c}", tag="o")
                d_ps = dps.tile([1, 512], F32, name=f"d{h}_{c}", tag="d")
                pts = [None] * npair

                def emit_scores(i):
                    ge, go, o0e, o0o = seq[i]
                    st = stps.tile([P, 1024], F32, name=f"st{h}_{c}_{i}", tag="st")
                    for j, (g, o0) in enumerate(((ge, o0e), (go, o0o))):
                        nc.tensor.matmul(
                            st[:, 512 * j + o0 : 512 * (j + 1)],
                            kt[h][:, P * g : P * (g + 1)],
                            qt[h][:, 512 * c + o0 : q_hi],
                            start=True, stop=True,
                        )
                    pt = ptp.tile([P, 1024], BF16, name=f"pt{h}_{c}_{i}", tag="pt")
                    nc.scalar.activation(pt[:, o0e:1024], st[:, o0e:1024], EXP)
                    if o0o:
                        nc.vector.memset(pt[:, 512 : 512 + o0o], 0.0)
                    if i in (2, 3):  # diagonal pair: triangular masks
                        nc.vector.tensor_mul(
                            pt[:, o0e : o0e + P], pt[:, o0e : o0e + P], tri[:]
                        )
                        nc.vector.tensor_mul(
                            pt[:, 512 + o0o : 512 + o0o + P],
                            pt[:, 512 + o0o : 512 + o0o + P],
                            tri[:],
                        )
                    pts[i] = pt

                def emit_pv(i):
                    ge, go, o0e, o0o = seq[i]
                    pt = pts[i]
                    for j, (g, o0) in enumerate(((ge, o0e), (go, o0o))):
                        first = i == 0 and j == 0
                        last = i == npair - 1 and j == 1
                        psl = slice(512 * j + o0, 512 * (j + 1))
                        nc.tensor.matmul(
                            o_ps[:, o0:512], vv[g][:, hsl], pt[:, psl],
                            start=first, stop=last,
                        )
                        nc.tensor.matmul(
                            d_ps[:, o0:512], ones[:], pt[:, psl],
                            start=first, stop=last,
                        )

                emit_scores(0)
                emit_scores(1)
                for i in range(npair):
                    emit_pv(i)
                    if i + 2 < npair:
                        emit_scores(i + 2)

                # normalize + fp8 hi/lo split of A^T
                rec = bcp.tile([1, 512], F32, name=f"rec{h}_{c}", tag="rec")
                nc.vector.reciprocal(rec[:], d_ps[:])
                bc = bcp.tile([P, 512], F32, name=f"bc{h}_{c}", tag="bc")
                nc.gpsimd.partition_broadcast(bc[:], rec[:])
                t = bcp.tile([P, 512], F32, name=f"t{h}_{c}", tag="t")
                nc.vector.tensor_mul(t[:], o_ps[:], bc[:])
                ah_sl = a2h[h // 2][:, h % 2, 512 * c : q_hi]
                nc.vector.tensor_copy(ah_sl, t[:])
                nc.vector.tensor_sub(
                    a2l[h // 2][:, h % 2, 512 * c : q_hi], t[:], ah_sl
                )

            def outproj(m, c):
                csl = slice(512 * c, 512 * (c + 1))
                op = aux.tile([P, 512], F32, name=f"op{m}_{c}", tag=f"op{m % 2}")
                wo = wo_t[m]
                for p2 in range(4):
                    ah_sl = a2h[p2][:, :, csl]
                    al_sl = a2l[p2][:, :, csl]
                    for k, (ws, rhs) in enumerate(
                        ((0, ah_sl), (0, al_sl), (1, ah_sl))
                    ):
                        nc.tensor.matmul(
                            op[:], wo[:, ws, p2], rhs,
                            start=(p2 == 0 and k == 0),
                            stop=(p2 == 3 and k == 2),
                            perf_mode=DR,
                        )
                ob = outs.tile([P, 512], F16, name=f"ob{m}_{c}", tag="ob")
                nc.vector.tensor_scalar_mul(ob[:], op[:], 1.0 / SIG_O)
                nc.sync.dma_start(prm["outT"][P * m : P * (m + 1), csl], ob[:])

            for h in range(HLOC):
                att(h, 0)
            for h in range(HLOC):
                att(h, 1)
                outproj(2 * h, 0)
                outproj(2 * h + 1, 0)
            for m in range(16):
                outproj(m, 1)


def build():
    nc = bacc.Bacc(None, target_bir_lowering=False)
    prm = {}
    for n, shape, dt in (
        ("xq", [KP, P, 2, 2, SQ], F8),
        ("xk", [KP, P, 2, 2, SQ], F8),
        ("xv", [KP, P, 2, 2, SQ], F8),
        ("wq", [KP, P, 2, 2, DH], F8),
        ("wk", [KP, P, 2, 2, DH], F8),
        ("wv", [KP, P, 2, 2, DH], F8),
        ("bv8", [1, DH], F8),
        ("wo", [16, P, 2, 4, 2, P], F8),
        ("ckt", [DH, SC], BF16),
        ("cv", [SC, DH], BF16),
        ("tri", [P, P], BF16),
        ("bq", [P, HLOC], F32),
        ("bk", [P, HLOC], F32),
    ):
        prm[n] = nc.declare_dram_parameter(n, shape, dt, isOutput=False)
    prm["outT"] = nc.declare_dram_parameter("outT", [D, SQ], F16, isOutput=True)
    with tile.TileContext(nc) as tc:
        _emit(tc, nc, prm)
    nc.compile()
    return nc


def _split8(a, sigma):
    hi = (a * sigma).astype(E4)
    lo = (a * sigma - hi.astype(np.float32)).astype(E4)
    return hi, lo


def _pack_x(x):
    """[SQ, D] -> fp8 hi/lo packed [KP, P, 2(hl), 2(j), SQ]."""
    xt = np.ascontiguousarray(x.T)  # [D, SQ]
    hi, lo = _split8(xt, 1.0)
    hi = hi.reshape(KP, 2, P, SQ).transpose(0, 2, 1, 3)
    lo = lo.reshape(KP, 2, P, SQ).transpose(0, 2, 1, 3)
    return np.ascontiguousarray(np.stack([hi, lo], axis=2))


def _pack_w(wT, sigma):
    """[D, DH] (pre-transposed W) -> [KP, P, 2, 2, DH] fp8 hi/lo."""
    hi, lo = _split8(wT, sigma)
    hi = hi.reshape(KP, 2, P, DH).transpose(0, 2, 1, 3)
    lo = lo.reshape(KP, 2, P, DH).transpose(0, 2, 1, 3)
    return np.ascontiguousarray(np.stack([hi, lo], axis=2))


def make_in_maps(query, key, value, cached_k, cached_v, Wq, bq, Wk, bk, Wv, bv, Wo, bo):
    s = float(np.sqrt(HD))
    tri = np.triu(np.ones((P, P), dtype=np.float32)).astype(BF)

    in_maps = []
    for c in range(NCORES):
        b, h2 = c // 2, c % 2
        hs = slice(DH * h2, DH * (h2 + 1))
        wo_s = np.ascontiguousarray(Wo[:, hs].T)  # [DH, D] rows = dh in
        woh, wol = _split8(wo_s, SIG_O)
        # [dh=(p2,j,p), dout=(m,ms)] -> [m, p, hl, p2, j, ms]
        woh = woh.reshape(4, 2, P, 16, P).transpose(3, 2, 0, 1, 4)
        wol = wol.reshape(4, 2, P, 16, P).transpose(3, 2, 0, 1, 4)
        wo_pk = np.ascontiguousarray(np.stack([woh, wol], axis=2))

        in_maps.append(
            {
                "xq": _pack_x(query[b]),
                "xk": _pack_x(key[b]),
                "xv": _pack_x(value[b]),
                "wq": _pack_w(np.ascontiguousarray(Wq[hs].T) / s, SIG_Q),
                "wk": _pack_w(np.ascontiguousarray(Wk[hs].T), SIG_K),
                "wv": _pack_w(np.ascontiguousarray(Wv[hs].T), SIG_V),
                "bv8": (bv[hs] * SIG_V).astype(E4).reshape(1, DH),
                "wo": wo_pk,
                "ckt": np.ascontiguousarray(cached_k[b][:, hs].T).astype(BF),
                "cv": np.ascontiguousarray(cached_v[b][:, hs]).astype(BF),
                "tri": tri,
                "bq": np.ascontiguousarray(
                    (bq[hs] / s).reshape(HLOC, P).T
                ).astype(np.float32),
                "bk": np.ascontiguousarray(
                    bk[hs].reshape(HLOC, P).T
                ).astype(np.float32),
            }
        )
    return in_maps


_NC_CACHE = []


def get_nc():
    if not _NC_CACHE:
        _NC_CACHE.append(build())
    return _NC_CACHE[0]


def assemble(results, bo):
    out = np.empty((4, SQ, D), dtype=np.float32)
    for b in range(4):
        acc = results[2 * b]["outT"].astype(np.float32) + results[2 * b + 1][
            "outT"
        ].astype(np.float32)
        out[b] = acc.T + bo[None, :]
    return out


def kernel(query, key, value, cached_k, cached_v, Wq, bq, Wk, bk, Wv, bv, Wo, bo):
    query = np.asarray(query, dtype=np.float32)
    key = np.asarray(key, dtype=np.float32)
    value = np.asarray(value, dtype=np.float32)
    cached_k = np.asarray(cached_k, dtype=np.float32)
    cached_v = np.asarray(cached_v, dtype=np.float32)
    Wq, bq = np.asarray(Wq, np.float32), np.asarray(bq, np.float32)
    Wk, bk = np.asarray(Wk, np.float32), np.asarray(bk, np.float32)
    Wv, bv = np.asarray(Wv, np.float32), np.asarray(bv, np.float32)
    Wo, bo = np.asarray(Wo, np.float32), np.asarray(bo, np.float32)

    nc = get_nc()
    in_maps = make_in_maps(
        query, key, value, cached_k, cached_v, Wq, bq, Wk, bk, Wv, bv, Wo, bo
    )
    res = run_bass_kernel_spmd(nc, in_maps, list(range(NCORES)))
    return assemble(res.results, bo)


# revision 18
# speedup vs baseline: 1.5639x; 1.0774x over previous
"""KV-cached multi-head attention on 8 Trainium2 NeuronCores.

Sharding: 4-way batch (data parallel) x 2-way heads (tensor parallel).
Core c handles batch b = c//2 and head-half h2 = c%2 (8 of 16 heads).
Each core: Q/K/V projections (column-sharded), 8-head causal attention
against the concatenated KV cache, and a row-sharded out-projection
partial. The two partials per batch are summed on the host (+ bo).

v2 kernel layout:
  - Q/K/V projections and the out-projection run as fp8e4m3 DoubleRow
    matmuls (256-wide contraction per instruction, 0.5 cyc/row) with a
    3-term error split: x = xh + xl (both fp8), W = wh + wl;
    acc = (xh+xl)@wh + xh@wl. Host pre-scales W by a power of two so
    fp8 quantization stays in the normal range; the PSUM->SBUF copy
    (DVE tensor_scalar) undoes the scale and adds the bias.
  - V projection carries its bias as an fp8 ones-row contraction pair
    built from tiny on-device tiles.
  - Attention (scores, exp, PV, softmax denominator) stays in bf16:
    S^T = K^T.T @ Q^T per 128-key tile (no transposes needed), exp on
    ACT over 2-tile pairs, O^T = V.T @ P^T, denominator via ones-column
    matmul. Causality handled structurally per 512-query chunk with
    triangular-mask multiplies on the 4 diagonal tiles; diagonal
    scores/PV/ones matmuls stream only the live query range (full-width
    pairs are ordered first/last so PSUM start/stop cover all columns).
  - A = O/d is split to fp8 hi+lo on DVE for the fp8 out-projection;
    out-projection (c=0) is interleaved into attention (c=1) to keep
    the PE fed while ACT works on exps.
"""

import sys

sys.path.insert(0, "/opt/trn_rl_repo")

import numpy as np
import ml_dtypes

import concourse.bass as bass  # noqa: F401  (registers AP types)
import concourse.mybir as mybir
import concourse.tile as tile
from concourse import bacc
from concourse.bass_utils import run_bass_kernel_spmd

F32 = mybir.dt.float32
F16 = mybir.dt.float16
BF16 = mybir.dt.bfloat16
F8 = mybir.dt.float8e4
BF = ml_dtypes.bfloat16
E4 = ml_dtypes.float8_e4m3

D = 2048          # model dim
SQ = 1024         # new tokens per batch
SC = 1024         # cached tokens
SKV = SC + SQ     # total keys
HD = 128          # head dim
HLOC = 8          # heads per core
DH = HLOC * HD    # per-core projected dim (1024)
KP = 8            # fp8 DoubleRow contraction pairs (8 * 256 = 2048)
NCORES = 8
P = 128

SIG_Q = 512.0     # host pre-scale on Wq/sqrt(hd)
SIG_K = 64.0
SIG_V = 64.0
SIG_O = 64.0

EXP = mybir.ActivationFunctionType.Exp
DR = mybir.MatmulPerfMode.DoubleRow
MULT = mybir.AluOpType.mult
ADD = mybir.AluOpType.add

TERMS = ((0, 0), (1, 0), (0, 1))  # (x hi/lo, w hi/lo) split terms


def _pair_seq(c):
    """Pair schedule for query chunk c: [(g_even, g_odd, o0_even, o0_odd)].
    Full-width pairs first and last so o_ps/d_ps start/stop cover all
    columns; diagonal pairs (reduced query range) in the middle."""
    n_full = 8 + 4 * c            # full 128-key tiles
    full = [(2 * i, 2 * i + 1, 0, 0) for i in range(n_full // 2)]
    diag = [
        (n_full, n_full + 1, 0, 128),
        (n_full + 2, n_full + 3, 256, 384),
    ]
    return full[:2] + diag + full[2:]


def _emit(tc, nc, prm):
    with tc.tile_pool(name="res", bufs=1) as res:
        qt = [res.tile([P, SQ], BF16, name=f"qt{h}", tag=f"qt{h}") for h in range(HLOC)]
        kta = res.tile([P, HLOC, SKV], BF16, name="kta", tag="kta")
        kt = [kta[:, h, :] for h in range(HLOC)]
        vvc = res.tile([P, 8, DH], BF16, name="vvc", tag="vvc")
        vvn = [res.tile([P, DH], BF16, name=f"vv{t}", tag=f"vv{t}") for t in range(8)]
        vv = [vvc[:, t, :] for t in range(8)] + [v[:] for v in vvn]
        a2h = [res.tile([P, 2, SQ], F8, name=f"a2h{p}", tag=f"a2h{p}") for p in range(4)]
        a2l = [res.tile([P, 2, SQ], F8, name=f"a2l{p}", tag=f"a2l{p}") for p in range(4)]
        tri = res.tile([P, P], BF16, name="tri", tag="tri")
        ones = res.tile([P, 1], BF16, name="ones", tag="ones")
        bqv = res.tile([P, HLOC], F32, name="bqv", tag="bqv")
        bkv = res.tile([P, HLOC], F32, name="bkv", tag="bkv")
        xb = res.tile([P, 2, P], F8, name="xb", tag="xb")
        wvb = res.tile([P, 2, DH], F8, name="wvb", tag="wvb")

        def load_xw(pool, pfx, src_x, src_w, kps):
            xs_t, ws_t = [], []
            for kp in kps:
                x = pool.tile([P, 2, 2, SQ], F8, name=f"x{pfx}{kp}", tag=f"x{pfx}{kp}")
                w = pool.tile([P, 2, 2, DH], F8, name=f"w{pfx}{kp}", tag=f"w{pfx}{kp}")
                if pfx == "v" and kp == 0:
                    # hi halves first so compute starts before lo lands
                    nc.sync.dma_start(x[:, 0], src_x[kp][:, 0])
                    nc.sync.dma_start(w[:, 0], src_w[kp][:, 0])
                    nc.sync.dma_start(x[:, 1], src_x[kp][:, 1])
                    nc.sync.dma_start(w[:, 1], src_w[kp][:, 1])
                else:
                    nc.sync.dma_start(x[:], src_x[kp])
                    nc.sync.dma_start(w[:], src_w[kp])
                xs_t.append(x)
                ws_t.append(w)
                if pfx == "v" and kp == 1:
                    nc.sync.dma_start(tri[:], prm["tri"][:])
                    nc.sync.dma_start(bqv[:], prm["bq"][:])
                    nc.sync.dma_start(bkv[:], prm["bk"][:])
                    nc.vector.memset(ones[:], 1.0)
                    nc.vector.memset(xb[:], 0.0)
                    nc.vector.memset(xb[0:1, 0, :], 1.0)
                    nc.vector.memset(wvb[:], 0.0)
                    nc.sync.dma_start(wvb[0:1, 0, :], prm["bv8"][:])
            return xs_t, ws_t

        def kq_proj(x_t, w_t, c, ps_pool, dest, sig, bias_t, nm):
            # m-outer / kp-inner: each head chunk's PSUM copy drains while
            # the next chunk's matmuls run
            csl = slice(512 * c, 512 * (c + 1))
            for m in range(8):
                ps = ps_pool.tile([P, 512], F32, name=f"p{nm}{c}_{m}", tag=f"ps{m}")
                msl = slice(P * m, P * (m + 1))
                for kp in range(KP):
                    for xs, ws in TERMS:
                        nc.tensor.matmul(
                            ps[:],
                            w_t[kp][:, ws, :, msl],
                            x_t[kp][:, xs, :, csl],
                            start=(kp == 0 and xs == 0 and ws == 0),
                            stop=(kp == KP - 1 and xs == 0 and ws == 1),
                            perf_mode=DR,
                        )
                nc.vector.tensor_scalar(
                    dest[m], ps[:], 1.0 / sig, bias_t[:, m : m + 1],
                    op0=MULT, op1=ADD,
                )

        # pre-pools hold the first 2 contraction pairs of the NEXT phase so
        # their DMAs land while the current phase computes (no barrier stall)
        with tc.tile_pool(name="preK", bufs=1) as preK:
            with tc.tile_pool(name="preQ", bufs=1) as preQ:
                # ---------------- V projection ----------------
                with (
                    tc.tile_pool(name="pjv", bufs=1) as pjv,
                    tc.tile_pool(name="pjps", bufs=1, space="PSUM") as pjps,
                ):
                    xv_t, wv_t = load_xw(pjv, "v", prm["xv"], prm["wv"], range(KP))
                    xkp_t, wkp_t = load_xw(preK, "kp", prm["xk"], prm["wk"], (0, 1))
                    for cd in range(2):
                        csl = slice(512 * cd, 512 * (cd + 1))
                        if cd == 0:
                            # DMA-paced first sweep: follow tile arrival order
                            ps_m = [
                                pjps.tile([P, 512], F32, name=f"vps0_{m}", tag=f"ps{m}")
                                for m in range(8)
                            ]
                            for kp in range(KP):
                                for xs, ws in TERMS:
                                    for m in range(8):
                                        msl = slice(P * m, P * (m + 1))
                                        nc.tensor.matmul(
                                            ps_m[m][:],
                                            xv_t[kp][:, xs, :, msl],
                                            wv_t[kp][:, ws, :, csl],
                                            start=(kp == 0 and xs == 0 and ws == 0
                                                   and m == 0),
                                            stop=False,
                                            perf_mode=DR,
                                        )
                            for m in range(8):
                                nc.tensor.matmul(
                                    ps_m[m][:], xb[:], wvb[:, :, csl],
                                    start=False, stop=True, perf_mode=DR,
                                )
                            for m in range(8):
                                nc.vector.tensor_scalar_mul(
                                    vvn[m][:, csl], ps_m[m][:], 1.0 / SIG_V
                                )
                        else:
                            # m-outer: PSUM copies drain progressively
                            for m in range(8):
                                ps = pjps.tile(
                                    [P, 512], F32, name=f"vps1_{m}", tag=f"ps{m}"
                                )
                                msl = slice(P * m, P * (m + 1))
                                for kp in range(KP):
                                    for xs, ws in TERMS:
                                        nc.tensor.matmul(
                                            ps[:],
                                            xv_t[kp][:, xs, :, msl],
                                            wv_t[kp][:, ws, :, csl],
                                            start=(kp == 0 and xs == 0 and ws == 0),
                                            stop=False,
                                            perf_mode=DR,
                                        )
                                nc.tensor.matmul(
                                    ps[:], xb[:], wvb[:, :, csl],
                                    start=False, stop=True, perf_mode=DR,
                                )
                                nc.vector.tensor_scalar_mul(
                                    vvn[m][:, csl], ps[:], 1.0 / SIG_V
                                )

                # ---------------- K projection ----------------
                with (
                    tc.tile_pool(name="pjk", bufs=1) as pjk,
                    tc.tile_pool(name="pjps2", bufs=1, space="PSUM") as pjps2,
                ):
                    xk_t, wk_t = load_xw(pjk, "k", prm["xk"], prm["wk"], range(2, KP))
                    xqp_t, wqp_t = load_xw(preQ, "qp", prm["xq"], prm["wq"], (0, 1))
                    xk_t = xkp_t + xk_t
                    wk_t = wkp_t + wk_t
                    for c in range(2):
                        kq_proj(
                            xk_t, wk_t, c, pjps2,
                            [kt[m][:, SC + 512 * c : SC + 512 * c + 512]
                             for m in range(8)],
                            SIG_K, bkv, "k",
                        )

                # ---------------- Q projection ----------------
                with (
                    tc.tile_pool(name="pjq", bufs=1) as pjq,
                    tc.tile_pool(name="pjps3", bufs=1, space="PSUM") as pjps3,
                ):
                    xq_t, wq_t = load_xw(pjq, "q", prm["xq"], prm["wq"], range(2, KP))
                    xq_t = xqp_t + xq_t
                    wq_t = wqp_t + wq_t
                    # cached KV + out-proj weights: batched DMAs, needed at
                    # attention; emitted here so transfers overlap Q compute
                    nc.sync.dma_start(
                        kta[:, :, 0:SC], prm["ckt"].rearrange("(h p) s -> p h s", p=P)
                    )
                    nc.sync.dma_start(
                        vvc[:], prm["cv"].rearrange("(t p) d -> p t d", p=P)
                    )
                    for c in range(2):
                        kq_proj(
                            xq_t, wq_t, c, pjps3,
                            [qt[m][:, 512 * c : 512 * (c + 1)] for m in range(8)],
                            SIG_Q, bqv, "q",
                        )

        # ---------------- attention + out-projection ----------------
        with (
            tc.tile_pool(name="watt", bufs=1) as watt,
            tc.tile_pool(name="stps", bufs=2, space="PSUM") as stps,
            tc.tile_pool(name="ops", bufs=2, space="PSUM") as ops,
            tc.tile_pool(name="dps", bufs=1, space="PSUM") as dps,
            tc.tile_pool(name="aux", bufs=1, space="PSUM") as aux,
            tc.tile_pool(name="ptp", bufs=3) as ptp,
            tc.tile_pool(name="bcp", bufs=2) as bcp,
            tc.tile_pool(name="outs", bufs=4) as outs,
        ):
            woa = watt.tile([P, 16, 2, 4, 2, P], F8, name="woa", tag="woa")
            nc.sync.dma_start(
                woa[:], prm["wo"].rearrange("m p a b c d -> p m a b c d")
            )
            wo_t = [woa[:, m] for m in range(16)]

            COPY = mybir.ActivationFunctionType.Copy

            def outproj(m, c, pool, tag):
                csl = slice(512 * c, 512 * (c + 1))
                op = pool.tile([P, 512], F32, name=f"op{m}_{c}", tag=tag)
                wo = wo_t[m]
                for p2 in range(4):
                    ah_sl = a2h[p2][:, :, csl]
                    al_sl = a2l[p2][:, :, csl]
                    for k, (ws, rhs) in enumerate(
                        ((0, ah_sl), (0, al_sl), (1, ah_sl))
                    ):
                        nc.tensor.matmul(
                            op[:], wo[:, ws, p2], rhs,
                            start=(p2 == 0 and k == 0),
                            stop=(p2 == 3 and k == 2),
                            perf_mode=DR,
                        )
                ob = outs.tile([P, 512], F16, name=f"ob{m}_{c}", tag="ob")
                nc.scalar.activation(ob[:], op[:], COPY, scale=1.0 / SIG_O)
                nc.sync.dma_start(prm["outT"][P * m : P * (m + 1), csl], ob[:])

            def att_chunk(c, fillers):
                """Software-pipelined attention over all heads of query
                chunk c. `fillers` is a list of emit-callbacks (PE filler
                work, e.g. out-proj partials) popped one at a time at
                regular points in the pipeline."""
                fillers = list(fillers)
                seq = _pair_seq(c)
                npair = len(seq)
                q_hi = 512 * (c + 1)
                flat = [(h, i) for h in range(HLOC) for i in range(npair)]
                st_of = {}
                pt_of = {}
                o_of = {}
                d_of = {}

                def emit_scores(k):
                    h, i = flat[k]
                    ge, go, o0e, o0o = seq[i]
                    st = stps.tile([P, 1024], F32, name=f"st{h}_{c}_{i}", tag="st")
                    for j, (g, o0) in enumerate(((ge, o0e), (go, o0o))):
                        nc.tensor.matmul(
                            st[:, 512 * j + o0 : 512 * (j + 1)],
                            kt[h][:, P * g : P * (g + 1)],
                            qt[h][:, 512 * c + o0 : q_hi],
                            start=True, stop=True,
                        )
                    pt = ptp.tile([P, 1024], BF16, name=f"pt{h}_{c}_{i}", tag="pt")
                    nc.scalar.activation(pt[:, o0e:1024], st[:, o0e:1024], EXP)
                    if o0o:
                        nc.vector.memset(pt[:, 512 : 512 + o0o], 0.0)
                    if i in (2, 3):  # diagonal pair: triangular masks
                        nc.vector.tensor_mul(
                            pt[:, o0e : o0e + P], pt[:, o0e : o0e + P], tri[:]
                        )
                        nc.vector.tensor_mul(
                            pt[:, 512 + o0o : 512 + o0o + P],
                            pt[:, 512 + o0o : 512 + o0o + P],
                            tri[:],
                        )
                    pt_of[k] = pt

                def emit_pv(k):
                    h, i = flat[k]
                    ge, go, o0e, o0o = seq[i]
                    if i == 0:
                        o_of[h] = ops.tile([P, 512], F32, name=f"o{h}_{c}", tag="o")
                        d_of[h] = dps.tile([1, 512], F32, name=f"d{h}_{c}", tag="d")
                    pt = pt_of.pop(k)
                    hsl = slice(P * h, P * (h + 1))
                    for j, (g, o0) in enumerate(((ge, o0e), (go, o0o))):
                        first = i == 0 and j == 0
                        last = i == npair - 1 and j == 1
                        psl = slice(512 * j + o0, 512 * (j + 1))
                        nc.tensor.matmul(
                            o_of[h][:, o0:512], vv[g][:, hsl], pt[:, psl],
                            start=first, stop=last,
                        )
                        nc.tensor.matmul(
                            d_of[h][:, o0:512], ones[:], pt[:, psl],
                            start=first, stop=last,
                        )

                def emit_norm(h):
                    rec = bcp.tile([1, 512], F32, name=f"rec{h}_{c}", tag="rec")
                    nc.vector.reciprocal(rec[:], d_of[h][:])
                    bc = bcp.tile([P, 512], F32, name=f"bc{h}_{c}", tag="bc")
                    nc.gpsimd.partition_broadcast(bc[:], rec[:])
                    t = bcp.tile([P, 512], F32, name=f"t{h}_{c}", tag="t")
                    nc.vector.tensor_mul(t[:], o_of[h][:], bc[:])
                    ah_sl = a2h[h // 2][:, h % 2, 512 * c : q_hi]
                    nc.vector.tensor_copy(ah_sl, t[:])
                    nc.vector.tensor_sub(
                        a2l[h // 2][:, h % 2, 512 * c : q_hi], t[:], ah_sl
                    )

                emit_scores(0)
                emit_scores(1)
                for k, (h, i) in enumerate(flat):
                    emit_pv(k)
                    if k + 2 < len(flat):
                        emit_scores(k + 2)
                    if i == npair - 1:
                        emit_norm(h)
                    if k % 4 == 3 and fillers:
                        fillers.pop(0)()

            att_chunk(0, [])
            att_chunk(1, [
                (lambda m=m: outproj(m, 0, aux, "op")) for m in range(16)
            ])
            for m in range(16):
                outproj(m, 1, ops, "o")


def build():
    nc = bacc.Bacc(None, target_bir_lowering=False)
    prm = {}
    for n, shape, dt in (
        ("xq", [KP, P, 2, 2, SQ], F8),
        ("xk", [KP, P, 2, 2, SQ], F8),
        ("xv", [KP, P, 2, 2, SQ], F8),
        ("wq", [KP, P, 2, 2, DH], F8),
        ("wk", [KP, P, 2, 2, DH], F8),
        ("wv", [KP, P, 2, 2, DH], F8),
        ("bv8", [1, DH], F8),
        ("wo", [16, P, 2, 4, 2, P], F8),
        ("ckt", [DH, SC], BF16),
        ("cv", [SC, DH], BF16),
        ("tri", [P, P], BF16),
        ("bq", [P, HLOC], F32),
        ("bk", [P, HLOC], F32),
    ):
        prm[n] = nc.declare_dram_parameter(n, shape, dt, isOutput=False)
    prm["outT"] = nc.declare_dram_parameter("outT", [D, SQ], F16, isOutput=True)
    with tile.TileContext(nc) as tc:
        _emit(tc, nc, prm)
    nc.compile()
    return nc


def _split8(a, sigma):
    hi = (a * sigma).astype(E4)
    lo = (a * sigma - hi.astype(np.float32)).astype(E4)
    return hi, lo


def _pack_x(x):
    """[SQ, D] -> fp8 hi/lo packed [KP, P, 2(hl), 2(j), SQ]."""
    xt = np.ascontiguousarray(x.T)  # [D, SQ]
    hi, lo = _split8(xt, 1.0)
    hi = hi.reshape(KP, 2, P, SQ).transpose(0, 2, 1, 3)
    lo = lo.reshape(KP, 2, P, SQ).transpose(0, 2, 1, 3)
    return np.ascontiguousarray(np.stack([hi, lo], axis=2))


def _pack_w(wT, sigma):
    """[D, DH] (pre-transposed W) -> [KP, P, 2, 2, DH] fp8 hi/lo."""
    hi, lo = _split8(wT, sigma)
    hi = hi.reshape(KP, 2, P, DH).transpose(0, 2, 1, 3)
    lo = lo.reshape(KP, 2, P, DH).transpose(0, 2, 1, 3)
    return np.ascontiguousarray(np.stack([hi, lo], axis=2))


def make_in_maps(query, key, value, cached_k, cached_v, Wq, bq, Wk, bk, Wv, bv, Wo, bo):
    s = float(np.sqrt(HD))
    tri = np.triu(np.ones((P, P), dtype=np.float32)).astype(BF)

    in_maps = []
    for c in range(NCORES):
        b, h2 = c // 2, c % 2
        hs = slice(DH * h2, DH * (h2 + 1))
        wo_s = np.ascontiguousarray(Wo[:, hs].T)  # [DH, D] rows = dh in
        woh, wol = _split8(wo_s, SIG_O)
        # [dh=(p2,j,p), dout=(m,ms)] -> [m, p, hl, p2, j, ms]
        woh = woh.reshape(4, 2, P, 16, P).transpose(3, 2, 0, 1, 4)
        wol = wol.reshape(4, 2, P, 16, P).transpose(3, 2, 0, 1, 4)
        wo_pk = np.ascontiguousarray(np.stack([woh, wol], axis=2))

        in_maps.append(
            {
                "xq": _pack_x(query[b]),
                "xk": _pack_x(key[b]),
                "xv": _pack_x(value[b]),
                "wq": _pack_w(np.ascontiguousarray(Wq[hs].T) / s, SIG_Q),
                "wk": _pack_w(np.ascontiguousarray(Wk[hs].T), SIG_K),
                "wv": _pack_w(np.ascontiguousarray(Wv[hs].T), SIG_V),
                "bv8": (bv[hs] * SIG_V).astype(E4).reshape(1, DH),
                "wo": wo_pk,
                "ckt": np.ascontiguousarray(cached_k[b][:, hs].T).astype(BF),
                "cv": np.ascontiguousarray(cached_v[b][:, hs]).astype(BF),
                "tri": tri,
                "bq": np.ascontiguousarray(
                    (bq[hs] / s).reshape(HLOC, P).T
                ).astype(np.float32),
                "bk": np.ascontiguousarray(
                    bk[hs].reshape(HLOC, P).T
                ).astype(np.float32),
            }
        )
    return in_maps


_NC_CACHE = []


def get_nc():
    if not _NC_CACHE:
        _NC_CACHE.append(build())
    return _NC_CACHE[0]


def assemble(results, bo):
    out = np.empty((4, SQ, D), dtype=np.float32)
    for b in range(4):
        acc = results[2 * b]["outT"].astype(np.float32) + results[2 * b + 1][
            "outT"
        ].astype(np.float32)
        out[b] = acc.T + bo[None, :]
    return out


def kernel(query, key, value, cached_k, cached_v, Wq, bq, Wk, bk, Wv, bv, Wo, bo):
    query = np.asarray(query, dtype=np.float32)
    key = np.asarray(key, dtype=np.float32)
    value = np.asarray(value, dtype=np.float32)
    cached_k = np.asarray(cached_k, dtype=np.float32)
    cached_v = np.asarray(cached_v, dtype=np.float32)
    Wq, bq = np.asarray(Wq, np.float32), np.asarray(bq, np.float32)
    Wk, bk = np.asarray(Wk, np.float32), np.asarray(bk, np.float32)
    Wv, bv = np.asarray(Wv, np.float32), np.asarray(bv, np.float32)
    Wo, bo = np.asarray(Wo, np.float32), np.asarray(bo, np.float32)

    nc = get_nc()
    in_maps = make_in_maps(
        query, key, value, cached_k, cached_v, Wq, bq, Wk, bk, Wv, bv, Wo, bo
    )
    res = run_bass_kernel_spmd(nc, in_maps, list(range(NCORES)))
    return assemble(res.results, bo)
